# revision 1
# baseline (speedup 1.0000x reference)
"""Trainium2 Bass kernel for nn_LogisticMixture (discretized logistic mixture loss).

Contract: kernel(**inputs) takes FULL unsharded numpy inputs
  x      [128, 32, 32, 256] f32
  value  [128, 32, 32, 3]   f32 (integer pixel values 0..255)
  W_conv [256, 100]         f32
  b_conv [100]              f32
and returns the full [128] f32 output (per-image sum of mixture log-probs).

Strategy: pure data parallelism over the batch dim across 8 NeuronCores
(16 images / 16384 pixels per core). Host pre-transposes each core's pixel
block to x^T [256, 16384] (bf16) so the PE consumes it as the stationary
matmul operand with no on-chip transposes; the 1x1-conv params land in
PSUM directly in [pixel, 100] layout for the elementwise epilogue.

Epilogue math (validated to ~1e-5 rel vs the jax reference):
  p = (y + 0.5 - locs_t)/scales_t ; r = 1/scales_t ; m = p - r
  nlp_low  = softplus(-p) = u ; nlp_high = softplus(m) = v
  nlp_mid  = u + v - ln(1 - e^{-r})
  comp_lp  = -sum_c nlp ; mix_lp = lse_k(logits+comp_lp) - lse_k(logits)
with softplus(x) = max(x,0) + log1p(exp(-|x|)) built from the single
natural_log_exp_and_others activation-table set (one table load total).
Host passes vp2 = value - 127 so that yb = y + 0.5 - 127.5 = vp2 exactly.
"""
import sys
import os

for _p in ("/opt/trn_rl_repo", "/root/.axon_site/_ro/trn_rl_repo"):
    if os.path.isdir(_p) and _p not in sys.path:
        sys.path.append(_p)

import numpy as np
import ml_dtypes

import concourse.bass as bass
import concourse.mybir as mybir
import concourse.tile as tile
from concourse import bacc
from concourse.bass_utils import run_bass_kernel_spmd
import concourse.hw_specs as hw_specs

F32 = mybir.dt.float32
F16 = mybir.dt.float16
I8 = mybir.dt.int8
BF16 = mybir.dt.bfloat16
AL = mybir.AluOpType
AF = mybir.ActivationFunctionType

N_CORES = 8
D = 256
M = 100          # NUM_MIX * NUM_OUT
K = 10           # mixtures
C = 3            # channels
LN_EXP_C2 = float(127.5 * np.exp(-7.0))  # 127.5 * e^-7 folded into scales_t


def _force_single_act_table():
    """All ACT funcs used here (Exp, Ln, Abs, Relu) live in the
    natural_log_exp_and_others set. The default chooser flip-flops between
    exp/ln sets, reloading tables ~6x per macro-tile (~1.3us each). Empty
    every other set (keeping dict order so set ids stay aligned with
    act_info.json) so one table load serves the whole kernel."""
    if getattr(hw_specs, "_ant_single_set", False):
        return
    orig = hw_specs.get_activation_tables
    import functools

    @functools.cache
    def patched(arch):
        tabs = dict(orig(arch))
        keep = "natural_log_exp_and_others"
        if keep in tabs:
            tabs = {k: (v if k == keep else set()) for k, v in tabs.items()}
        return tabs

    hw_specs.get_activation_tables = patched
    bacc.get_activation_tables = patched
    hw_specs._ant_single_set = True


def _bc(ap_or_tile, offset, pattern):
    """Stride-0-capable AP on a tile: keep partition dim, replace free dims."""
    ap0 = ap_or_tile[:, :]
    return bass.AP(tensor=ap0.tensor, offset=ap0.offset + offset,
                   ap=[list(ap0.ap[0])] + [list(p) for p in pattern])


def build_program(pix=16384, with_bias=False):
    """Single-core SPMD program. pix must be a multiple of 2048."""
    TP = 2048                  # pixels per macro-tile (2 images)
    NT = pix // TP             # macro-tiles
    NS = 16                    # 128-px subtiles per macro-tile
    NQ = 4                     # psum tiles (quarters) per macro-tile
    NPT = pix // 128
    NIMG = pix // 1024

    _force_single_act_table()
    nc = bacc.Bacc("TRN2", target_bir_lowering=False, debug=False)

    xT_d = nc.dram_tensor("xT", [D, pix], BF16, kind="ExternalInput").ap()
    w_d = nc.dram_tensor("w", [D, M], BF16, kind="ExternalInput").ap()
    vp_d = nc.dram_tensor("vp", [128, NPT * C], F32, kind="ExternalInput").ap()
    if with_bias:
        bias_d = nc.dram_tensor("bias", [1, M], F32, kind="ExternalInput").ap()
    acc_d = nc.dram_tensor("acc", [128, NIMG], F32, kind="ExternalOutput").ap()

    with tile.TileContext(nc) as tc, \
            tc.tile_pool(name="const", bufs=1) as cpool, \
            tc.tile_pool(name="xin", bufs=3) as xpool, \
            tc.tile_pool(name="ps", bufs=6, space="PSUM") as pspool, \
            tc.tile_pool(name="ep", bufs=2) as ep:

        w_sb = cpool.tile([128, 2 * M], BF16)
        nc.sync.dma_start(out=w_sb, in_=bass.AP(
            tensor=w_d.tensor, offset=0, ap=[[M, 128], [128 * M, 2], [1, M]]))
        vp_sb = cpool.tile([128, NPT * C], F32)   # vp2 = value - 127, packed
        nc.sync.dma_start(out=vp_sb, in_=vp_d)
        acc = cpool.tile([128, NIMG], F32)
        if with_bias:
            bias_sb = cpool.tile([128, M], F32)
            nc.sync.dma_start(out=bias_sb, in_=bass.AP(
                tensor=bias_d.tensor, offset=0, ap=[[0, 128], [1, M]]))

        KC = K * C
        NKC = NS * KC            # 480
        NK = NS * K              # 160

        for j in range(NT):
            xt0 = xpool.tile([128, TP], BF16, tag="xt0")
            xt1 = xpool.tile([128, TP], BF16, tag="xt1")
            nc.sync.dma_start(out=xt0, in_=xT_d[0:128, j * TP:(j + 1) * TP])
            nc.sync.dma_start(out=xt1, in_=xT_d[128:256, j * TP:(j + 1) * TP])

            pp = []
            for h in range(NQ):
                t = pspool.tile([128, 4 * M], F32, tag="pp")
                pp.append(t)
                for s4 in range(4):
                    sub = h * 4 + s4
                    o = t[:, s4 * M:(s4 + 1) * M]
                    nc.tensor.matmul(o, xt0[:, sub * 128:(sub + 1) * 128],
                                     w_sb[:, 0:M], start=True, stop=False)
                    nc.tensor.matmul(o, xt1[:, sub * 128:(sub + 1) * 128],
                                     w_sb[:, M:2 * M], start=False, stop=True)
                if with_bias:
                    v4 = t[:, :].rearrange("p (s f) -> p s f", s=4)
                    nc.vector.tensor_tensor(
                        v4, v4, _bc(bias_sb, 0, [[0, 4], [1, M]]), AL.add)

            # ---- per-pixel values: vp2 = y - 127 for this macro-tile ----
            voff = j * NS * C
            vp_b = _bc(vp_sb, voff, [[C, NS], [0, K], [1, C]])      # [p,s,k,c]

            # q0 = -127.5*t0 = 0.5 - vp2_c0 ; q1 likewise on channel 1
            q0 = ep.tile([128, NS], F32, tag="q0")
            q1 = ep.tile([128, NS], F32, tag="q1")
            v3 = vp_sb[:, voff:voff + NS * C].rearrange("p (s c) -> p s c", c=C)
            nc.vector.tensor_scalar(q0, v3[:, :, 0], -1.0, 0.5, AL.mult, AL.add)
            nc.vector.tensor_scalar(q1, v3[:, :, 1], -1.0, 0.5, AL.mult, AL.add)
            # masks: y==0 <=> min(vp2+126,0)!=0 ; y==255 <=> max(vp2-127,0)!=0
            mlowb = ep.tile([128, NKC], I8, tag="mlowb")
            mhighb = ep.tile([128, NKC], I8, tag="mhighb")
            nc.vector.tensor_scalar(mlowb, vp_b, 126.0, 0.0, AL.add, AL.min)
            nc.vector.tensor_scalar(mhighb, vp_b, -127.0, 0.0, AL.add, AL.max)

            # ---- A = vp2 - 127.5*locs_m (fp16), scales path (f32) ----
            A = ep.tile([128, NKC], F16, tag="A")
            A4 = A[:, :].rearrange("p (s k c) -> p s k c", k=K, c=C)
            Xs = ep.tile([128, NKC], F32, tag="Xs")
            Rs = ep.tile([128, NKC], F32, tag="Rs")
            for h in range(NQ):
                f4 = pp[h][:, :].rearrange("p (s k f) -> p s k f", k=K, f=K)
                qs = slice(h * 120, (h + 1) * 120)
                A_h = A4[:, h * 4:(h + 1) * 4]
                for c in range(C):
                    vp_bhc = _bc(vp_sb, voff + h * 4 * C + c, [[C, 4], [0, K]])
                    nc.vector.scalar_tensor_tensor(
                        A_h[:, :, :, c], f4[:, :, :, 1 + c], -127.5,
                        vp_bhc, AL.mult, AL.add)
                q0bh = _bc(q0, h * 4, [[1, 4], [0, K]])
                q1bh = _bc(q1, h * 4, [[1, 4], [0, K]])
                h0 = ep.tile([128, 40], F16, tag="h0")
                h1 = ep.tile([128, 40], F16, tag="h1")
                h2 = ep.tile([128, 40], F16, tag="h2")
                nc.vector.tensor_tensor(h0, q0bh, f4[:, :, :, 7], AL.mult)
                nc.vector.tensor_tensor(h1, q0bh, f4[:, :, :, 8], AL.mult)
                nc.vector.tensor_tensor(h2, q1bh, f4[:, :, :, 9], AL.mult)
                nc.vector.tensor_tensor(A_h[:, :, :, 1], A_h[:, :, :, 1], h0, AL.add)
                h12 = ep.tile([128, 40], F16, tag="h12")
                nc.vector.tensor_tensor(h12, h1, h2, AL.add)
                nc.vector.tensor_tensor(A_h[:, :, :, 2], A_h[:, :, :, 2], h12, AL.add)
                nc.scalar.activation(Xs[:, qs], f4[:, :, :, 4:7], AF.Abs)
                nc.scalar.activation(Rs[:, qs], f4[:, :, :, 4:7], AF.Relu, scale=127.5)

            Es = ep.tile([128, NKC], F32, tag="Es")
            nc.scalar.activation(Es, Xs, AF.Exp, scale=-1.0)
            Ls = ep.tile([128, NKC], F32, tag="Ls")
            nc.scalar.activation(Ls, Es, AF.Ln, bias=1.0)
            sct = ep.tile([128, NKC], F32, tag="sct")
            nc.vector.affine_then_add(sct, Ls, Rs, 127.5, LN_EXP_C2)

            rr = ep.tile([128, NKC], F32, tag="rr")
            nc.vector.reciprocal_approx_fast(rr, sct)
            X = ep.tile([128, 3 * NKC], F16, tag="X")
            nc.gpsimd.tensor_copy(X[:, 2 * NKC:3 * NKC], rr)   # r as fp16
            r16 = X[:, 2 * NKC:3 * NKC]
            p = ep.tile([128, NKC], F16, tag="p")
            nc.vector.tensor_tensor(p, A, rr, AL.mult)
            m = ep.tile([128, NKC], F16, tag="m")
            nc.vector.tensor_tensor(m, p, r16, AL.subtract)
            mnp = ep.tile([128, NKC], F16, tag="mnp")
            mxm = ep.tile([128, NKC], F16, tag="mxm")
            nc.vector.tensor_scalar(mnp, p, 0.0, None, AL.min)
            nc.vector.tensor_scalar(mxm, m, 0.0, None, AL.max)
            # |p| = p - 2*min(p,0) ; |m| = 2*max(m,0) - m
            nc.vector.scalar_tensor_tensor(X[:, 0:NKC], mnp, -2.0, p,
                                           AL.mult, AL.add)
            nc.vector.scalar_tensor_tensor(X[:, NKC:2 * NKC], mxm, 2.0, m,
                                           AL.mult, AL.subtract)

            E = ep.tile([128, 3 * NKC], F32, tag="E")
            nc.scalar.activation(E, X, AF.Exp, scale=-1.0)
            nc.vector.tensor_scalar(E[:, 2 * NKC:3 * NKC], E[:, 2 * NKC:3 * NKC],
                                    -1.0, None, AL.mult)
            L = ep.tile([128, 3 * NKC], F16, tag="L")
            nc.scalar.activation(L, E, AF.Ln, bias=1.0)

            u = ep.tile([128, NKC], F16, tag="u")
            v = ep.tile([128, NKC], F16, tag="v")
            nc.vector.tensor_tensor(u, L[:, 0:NKC], mnp, AL.subtract)
            nc.vector.tensor_tensor(v, L[:, NKC:2 * NKC], mxm, AL.add)
            w1 = ep.tile([128, NKC], F16, tag="w1")
            nc.vector.tensor_tensor(w1, u, v, AL.add)
            w = ep.tile([128, NKC], F16, tag="w")
            nc.vector.tensor_tensor(w, w1, L[:, 2 * NKC:3 * NKC], AL.subtract)
            nc.vector.copy_predicated(w, mlowb, u)
            nc.vector.copy_predicated(w, mhighb, v)

            S = ep.tile([128, NK], F32, tag="S")
            nc.vector.reduce_sum(S, w[:, :].rearrange("p (sk c) -> p sk c", c=C),
                                 axis=mybir.AxisListType.X)
            z = ep.tile([128, NK], F32, tag="z")
            z3 = z[:, :].rearrange("p (s k) -> p s k", k=K)
            S3 = S[:, :].rearrange("p (s k) -> p s k", k=K)
            e2t = ep.tile([128, NK], F32, tag="e2t")
            e2t3 = e2t[:, :].rearrange("p (s k) -> p s k", k=K)
            for h in range(NQ):
                f4 = pp[h][:, :].rearrange("p (s k f) -> p s k f", k=K, f=K)
                nc.vector.scalar_tensor_tensor(
                    z3[:, h * 4:(h + 1) * 4], S3[:, h * 4:(h + 1) * 4],
                    -1.0, f4[:, :, :, 0], AL.mult, AL.add)
                nc.scalar.activation(e2t3[:, h * 4:(h + 1) * 4],
                                     f4[:, :, :, 0], AF.Exp)
            m1n = ep.tile([128, NS], F32, tag="m1n")
            nc.vector.tensor_reduce(m1n, z3, axis=mybir.AxisListType.X,
                                    op=AL.max, negate=True)
            zz = ep.tile([128, NK], F32, tag="zz")
            nc.vector.tensor_tensor(zz, z, _bc(m1n, 0, [[1, NS], [0, K]]), AL.add)
            e1t = ep.tile([128, NK], F32, tag="e1t")
            nc.scalar.activation(e1t, zz, AF.Exp)
            s12 = ep.tile([128, 2 * NS], F32, tag="s12")
            nc.vector.reduce_sum(s12[:, 0:NS],
                                 e1t[:, :].rearrange("p (s k) -> p s k", k=K),
                                 axis=mybir.AxisListType.X)
            nc.vector.reduce_sum(s12[:, NS:2 * NS], e2t3,
                                 axis=mybir.AxisListType.X)
            lse = ep.tile([128, 2 * NS], F32, tag="lse")
            nc.scalar.activation(lse, s12, AF.Ln)
            mix = ep.tile([128, NS], F32, tag="mix")
            nc.vector.tensor_tensor(mix, lse[:, 0:NS], lse[:, NS:2 * NS],
                                    AL.subtract)
            mix2 = ep.tile([128, NS], F32, tag="mix2")
            nc.vector.tensor_tensor(mix2, mix, m1n, AL.subtract)
            nc.vector.reduce_sum(acc[:, 2 * j:2 * j + 1], mix2[:, 0:8],
                                 axis=mybir.AxisListType.X)
            nc.vector.reduce_sum(acc[:, 2 * j + 1:2 * j + 2], mix2[:, 8:16],
                                 axis=mybir.AxisListType.X)

        nc.sync.dma_start(out=acc_d, in_=acc)

    nc.compile()
    return nc


_CACHE = {}


def _get_program(pix, with_bias):
    key = (pix, with_bias)
    if key not in _CACHE:
        _CACHE[key] = build_program(pix, with_bias)
    return _CACHE[key]


def pack_value(vf, per):
    """vf [per,3] raw 0..255 -> packed vp2 [128, per//128*3] = value-127."""
    return np.ascontiguousarray(
        (vf - 127.0).astype(np.float32)
        .reshape(per // 128, 128, C).transpose(1, 0, 2).reshape(128, -1))


def shard_inputs(x, value, W_conv, b_conv, n_cores=N_CORES):
    B = x.shape[0]
    pix_total = B * x.shape[1] * x.shape[2]
    per = pix_total // n_cores
    xf = np.ascontiguousarray(x.reshape(pix_total, D).astype(np.float32))
    vf = value.reshape(pix_total, C).astype(np.float32)
    w_bf = np.ascontiguousarray(W_conv.astype(ml_dtypes.bfloat16))
    with_bias = bool(np.any(b_conv))
    in_maps = []
    for i in range(n_cores):
        xT = np.ascontiguousarray(
            xf[i * per:(i + 1) * per].T.astype(ml_dtypes.bfloat16))
        vp = pack_value(vf[i * per:(i + 1) * per], per)
        mm = {"xT": xT, "w": w_bf, "vp": vp}
        if with_bias:
            mm["bias"] = b_conv.reshape(1, M).astype(np.float32)
        in_maps.append(mm)
    return in_maps, with_bias, per


def kernel(x, value, W_conv, b_conv):
    x = np.asarray(x)
    value = np.asarray(value)
    W_conv = np.asarray(W_conv)
    b_conv = np.asarray(b_conv)
    in_maps, with_bias, per = shard_inputs(x, value, W_conv, b_conv)
    nc = _get_program(per, with_bias)
    res = run_bass_kernel_spmd(nc, in_maps, list(range(N_CORES)))
    parts = []
    for i in range(N_CORES):
        acc = res.results[i]["acc"]
        parts.append(acc.astype(np.float64).sum(axis=0).astype(np.float32))
    return np.concatenate(parts)

